# revision 60
# baseline (speedup 1.0000x reference)
"""AttentionBlock (GroupNorm + 4-head self-attention + proj + residual) on 8 trn2 cores.

Sharding: data-parallel over batch (B=16 -> 2 per core). Each core runs the full
block on its 2 batch elements; no collectives.

Device pipeline per batch (all layouts chosen so no on-device transposes are needed):
  - GroupNorm stats via bn_stats/bn_aggr + tiny PE matmuls for the cross-partition
    group combine (gamma/beta folded into the qkv weights on host); inv_std via a
    DVE bit-trick rsqrt + Newton so the ACT engine never leaves the Exp table.
  - h cast to fp8 (e4m3); qkv / V^T / proj GEMMs run fp8 DoubleRow (2 MAC/cell/cyc)
    with weights pre-scaled x8 on host for e4m3 range, compensated in the PSUM evac.
    Weight fp8 casts run on the ACT engine, which is idle until the first exp.
  - Q,K evac'd to f32r, so the S^T matmuls are full precision (their contraction
    is only d=64, so DoubleRow could not speed them up anyway).
  - Scores computed transposed: S^T[m,n] = K^T Q; softmax's sum runs over the
    PSUM partition axis via all-ones stationary columns packed next to V.
  - exp(S-3) on ACT straight out of PSUM into fp8 m-chunk-pair tiles (S bounded
    ~|8|; the uniform e^-3 scale cancels in the softmax normalization).
  - AV+Z as fp8 DoubleRow over m-chunk pairs: [V_h|1] stationary emits O and the
    denominator Z replicated across partitions in one pass.
  - Normalization reads O and Z directly from PSUM (reciprocal at base partition
    0 only -- reciprocal_approx_fast mis-executes at base 64); Z moved across
    partitions by small SBUF-SBUF DMAs; output quantized to fp8 p-pair tiles.
  - proj fp8 DoubleRow per nch half (starts after 2 of 4 units) + residual from
    x with the proj bias pre-added on the DVE (no rank-1 bias matmuls).
"""

import numpy as np
from contextlib import ExitStack

import concourse.bass as bass
import concourse.bacc as bacc
import concourse.tile as tile
import concourse.mybir as mybir
from concourse.bass_utils import run_bass_kernel_spmd

F32 = mybir.dt.float32
F32R = mybir.dt.float32r
BF16 = mybir.dt.bfloat16
FP8 = mybir.dt.float8e4
I32 = mybir.dt.int32
DR = mybir.MatmulPerfMode.DoubleRow

B, C, HH, WW = 16, 256, 32, 32
N = HH * WW           # 1024 spatial positions
NH = 4                # heads
D = C // NH           # 64 head dim
G = 32                # groups
EPS = 1e-5
NCORES = 8
BL = B // NCORES      # batches per core

RSQRT_MAGIC = 0x5F3759DF
QKV_SCALE = 0.125             # weights stored x8
K_SCALE = 0.125 * 0.125       # extra d^-0.5 fold for K


def build_bass():
    nc = bacc.Bacc("TRN2", target_bir_lowering=False, debug=False)

    x_d = nc.dram_tensor("x", [BL, C, N], F32, kind="ExternalInput").ap()
    wqk_d = nc.dram_tensor("wqk8", [128, 1024], F32, kind="ExternalInput").ap()
    wv_d = nc.dram_tensor("wv8", [128, 512], F32, kind="ExternalInput").ap()
    wp_d = nc.dram_tensor("wp8", [128, 512], F32, kind="ExternalInput").ap()
    bqk_d = nc.dram_tensor("bqk", [4, 128], F32, kind="ExternalInput").ap()
    bv_d = nc.dram_tensor("bv", [1, 256], F32R, kind="ExternalInput").ap()
    bp_d = nc.dram_tensor("bp2", [2, 128], F32, kind="ExternalInput").ap()
    gmap_d = nc.dram_tensor("gmap", [128, 16], F32, kind="ExternalInput").ap()
    gexp_d = nc.dram_tensor("gexp", [16, 128], F32, kind="ExternalInput").ap()
    y_d = nc.dram_tensor("y", [BL, C, N], F32, kind="ExternalOutput").ap()

    Exp = mybir.ActivationFunctionType.Exp
    Ident = mybir.ActivationFunctionType.Identity
    mult = mybir.AluOpType.mult
    sub = mybir.AluOpType.subtract
    add = mybir.AluOpType.add
    asr = mybir.AluOpType.arith_shift_right

    with tile.TileContext(nc) as tc, ExitStack() as ctx:
        consts = ctx.enter_context(tc.tile_pool(name="consts", bufs=1))
        xpool = ctx.enter_context(tc.tile_pool(name="xp", bufs=1))
        xbpool = ctx.enter_context(tc.tile_pool(name="xbp", bufs=1))
        hpool = ctx.enter_context(tc.tile_pool(name="hp", bufs=1))
        qkpool = ctx.enter_context(tc.tile_pool(name="qkp", bufs=1))
        vtpool = ctx.enter_context(tc.tile_pool(name="vtp", bufs=1))
        opool = ctx.enter_context(tc.tile_pool(name="op", bufs=1))
        gnpool = ctx.enter_context(tc.tile_pool(name="gnp", bufs=2))
        expool = ctx.enter_context(tc.tile_pool(name="exp", bufs=4))
        rzpool = ctx.enter_context(tc.tile_pool(name="rzp", bufs=3))
        outpool = ctx.enter_context(tc.tile_pool(name="outp", bufs=4))
        ps_big = ctx.enter_context(tc.tile_pool(name="psb", bufs=3, space="PSUM"))
        ps_o = ctx.enter_context(tc.tile_pool(name="pso", bufs=2, space="PSUM"))

        x_sb = [[None] * 2 for _ in range(BL)]
        xb_sb = [[None] * 2 for _ in range(BL)]
        h_sb = [None] * BL            # fp8 ct-pair tiles [128, 2*1024]
        qk_sb = [[None] * 4 for _ in range(BL)]   # f32r: q01 q23 k01 k23
        vt_sb = [[None] * 4 for _ in range(BL)]   # fp8 m-chunk pair tiles
        o_sb = [None] * BL            # fp8 p-pair tiles [128, 2*1024]

        # --- x first (GroupNorm gates everything), split for earlier stats ---
        for b in range(BL):
            for ct in range(2):
                xt = xpool.tile([128, N], F32, tag=f"x{b}{ct}", name=f"x{b}{ct}")
                x_sb[b][ct] = xt
                nc.sync.dma_start(xt[:, 0:512], x_d[b, ct * 128:(ct + 1) * 128, 0:512])
                nc.sync.dma_start(xt[:, 512:1024], x_d[b, ct * 128:(ct + 1) * 128, 512:1024])

        # --- constants / weights into SBUF (fp8 casts on the idle ACT) ------
        gmap_sb = consts.tile([128, 16], F32, tag="gmap")
        nc.sync.dma_start(gmap_sb[:], gmap_d[:])
        gexp_sb = consts.tile([16, 128], F32, tag="gexp")
        nc.sync.dma_start(gexp_sb[:], gexp_d[:])
        bqk_sb = consts.tile([128, 4], F32, tag="bqk")
        nc.sync.dma_start(bqk_sb[:], bqk_d.transpose([1, 0]))
        bp_sb = consts.tile([128, 2], F32, tag="bp")
        nc.sync.dma_start(bp_sb[:], bp_d.transpose([1, 0]))
        bv_sb = consts.tile([1, 256], F32R, tag="bv")
        nc.sync.dma_start(bv_sb[:], bv_d[:])
        wqk_f = consts.tile([128, 1024], F32, tag="wqk_f")
        nc.sync.dma_start(wqk_f[:, 0:512], wqk_d[:, 0:512])
        nc.sync.dma_start(wqk_f[:, 512:1024], wqk_d[:, 512:1024])
        wv_f = consts.tile([128, 512], F32, tag="wv_f")
        nc.sync.dma_start(wv_f[:], wv_d[:])
        wp_f = consts.tile([128, 512], F32, tag="wp_f")
        nc.sync.dma_start(wp_f[:], wp_d[:])

        ones_f32 = consts.tile([128, 512], F32, tag="ones_f32")
        nc.vector.memset(ones_f32[:], 1.0)
        neg3 = consts.tile([128, 1], F32, tag="neg3")
        nc.vector.memset(neg3[:], -3.0)
        ones_sb = consts.tile([128, 512], F32R, tag="ones")
        nc.vector.tensor_copy(ones_sb[:], ones_f32[:])
        wqk8 = consts.tile([128, 1024], FP8, tag="wqk8")
        nc.scalar.copy(wqk8[:], wqk_f[:])
        wv8 = consts.tile([128, 512], FP8, tag="wv8")
        nc.scalar.copy(wv8[:], wv_f[:])
        wp8 = consts.tile([128, 512], FP8, tag="wp8")
        nc.scalar.copy(wp8[:], wp_f[:])
        wqk2 = wqk8[:].rearrange("p (s o) -> p s o", s=2)
        wv2 = wv8[:].rearrange("p (s o) -> p s o", s=2)
        wp2 = wp8[:].rearrange("p (s o) -> p s o", s=2)

        # bv broadcast to all partitions via a rank-1 matmul (done once)
        pbv = ps_o.tile([128, 256], F32, tag="o", name="pbv")
        nc.tensor.matmul(pbv[:], lhsT=ones_sb[0:1, 0:128], rhs=bv_sb[0:1, :],
                         start=True, stop=True)
        bvb = consts.tile([128, 256], F32, tag="bvb")
        nc.scalar.copy(bvb[:], pbv[:])

        # ====== Phase A: GroupNorm, batched 4-wide across (b, ct) tiles ======
        units4 = [(b, ct) for b in range(BL) for ct in range(2)]
        bn6 = gnpool.tile([128, 48], F32, tag="bn6")
        mva = gnpool.tile([128, 8], F32, tag="mva")  # (mean, var) x 4 units
        for u4, (b, ct) in enumerate(units4):
            xt = x_sb[b][ct]
            nc.vector.bn_stats(bn6[:, 12 * u4:12 * u4 + 6], xt[:, 0:512])
            nc.vector.bn_stats(bn6[:, 12 * u4 + 6:12 * u4 + 12], xt[:, 512:1024])
            nc.vector.bn_aggr(mva[:, 2 * u4:2 * u4 + 2], bn6[:, 12 * u4:12 * u4 + 12])
        mva2 = mva[:].rearrange("p (u c) -> p u c", c=2)
        m2a = gnpool.tile([128, 4], F32, tag="m2a")
        nc.vector.tensor_mul(m2a[:], mva2[:, :, 0], mva2[:, :, 0])
        # group-combine via matmul, then a short DVE-only chain:
        # s = rsqrt(var+eps) via bit trick + one Newton step; t = mean*s.
        psg = ps_o.tile([16, 12], F32, tag="o", name="psg")
        nc.tensor.matmul(psg[:, 0:8], lhsT=gmap_sb[:], rhs=mva[:], start=True,
                         stop=True, skip_group_check=True)
        nc.tensor.matmul(psg[:, 8:12], lhsT=gmap_sb[:], rhs=m2a[:], start=True,
                         stop=True, skip_group_check=True)
        g = gnpool.tile([16, 20], F32, tag="g")
        nc.vector.tensor_copy(g[:, 0:12], psg[:])
        g2 = g[:, 0:8].rearrange("p (u c) -> p u c", c=2)
        # var_g + eps = (E[var] + eps) + E[mean^2] - mean_g^2
        nc.vector.scalar_tensor_tensor(g[:, 12:16], g2[:, :, 1], EPS,
                                       g[:, 8:12], add, add)
        nc.vector.tensor_mul(g[:, 16:20], g2[:, :, 0], g2[:, :, 0])
        nc.vector.tensor_sub(g[:, 12:16], g[:, 12:16], g[:, 16:20])
        gi = g[:].bitcast(I32)
        nc.vector.tensor_scalar(gi[:, 16:20], gi[:, 12:16], 1, None, asr)
        nc.vector.tensor_scalar(gi[:, 16:20], gi[:, 16:20], -1, RSQRT_MAGIC,
                                mult, add)
        sg = gnpool.tile([16, 8], F32, tag="sg")
        s2 = sg[:].rearrange("p (u c) -> p u c", c=2)
        nc.vector.tensor_mul(g[:, 8:12], g[:, 16:20], g[:, 16:20])
        nc.vector.tensor_mul(g[:, 8:12], g[:, 8:12], g[:, 12:16])
        nc.vector.tensor_scalar(g[:, 8:12], g[:, 8:12], -0.5, 1.5, mult, add)
        nc.vector.tensor_mul(s2[:, :, 0], g[:, 16:20], g[:, 8:12])
        nc.vector.tensor_mul(s2[:, :, 1], g2[:, :, 0], s2[:, :, 0])
        psc = ps_o.tile([128, 8], F32, tag="o", name="psc")
        nc.tensor.matmul(psc[:], lhsT=gexp_sb[:], rhs=sg[:], start=True, stop=True)
        st = gnpool.tile([128, 8], F32, tag="st")
        nc.vector.tensor_copy(st[:], psc[:])
        # PE warm-up: the clock needs ~3.4us of sustained activity; these run
        # while the DVE finishes the GN chain + h applies.
        for w in range(14):
            nc.tensor.matmul(pbv[:], lhsT=ones_sb[0:1, 0:128],
                             rhs=bv_sb[0:1, :], start=True, stop=True)
        for u4, (b, ct) in enumerate(units4):
            # h = x * s - t  (gamma/beta already folded into W/b on host)
            if ct == 0:
                h_sb[b] = hpool.tile([128, 2 * N], FP8, tag=f"h{b}", name=f"h{b}")
            nc.vector.tensor_scalar(h_sb[b][:, N * ct:N * ct + N], x_sb[b][ct][:],
                                    st[:, 2 * u4:2 * u4 + 1],
                                    st[:, 2 * u4 + 1:2 * u4 + 2], mult, sub)

        for b in range(BL):
            h2 = h_sb[b][:].rearrange("p (s n) -> p s n", s=2)
            # Q,K GEMM (fp8 DoubleRow over the 2 c-blocks): ot 0=q01 1=q23
            # 2=k01 3=k23; the evac rescales the x8 weights and adds the bias.
            for ot in (2, 0, 3, 1):
                pq = ps_big.tile([128, N], F32, tag="big")
                for nch in range(2):
                    ns = slice(nch * 512, (nch + 1) * 512)
                    nc.tensor.matmul(pq[:, ns],
                                     lhsT=wqk2[:, :, ot * 128:(ot + 1) * 128],
                                     rhs=h2[:, :, ns], start=True, stop=True,
                                     perf_mode=DR)
                qk = qkpool.tile([128, N], F32R, tag=f"qk{b}{ot}")
                qk_sb[b][ot] = qk
                sc = K_SCALE if ot >= 2 else QKV_SCALE
                if b == 0:
                    # batch 0's evacs on the (still idle) ACT engine so the
                    # DVE builds vt tiles in parallel -> earlier first S;
                    # Identity shares the Exp table (no table reload).
                    nc.scalar.activation(qk[:], pq[:], Ident,
                                         bias=bqk_sb[:, ot:ot + 1], scale=sc)
                else:
                    nc.vector.tensor_scalar(qk[:], pq[:], sc,
                                            bqk_sb[:, ot:ot + 1], mult, add)

            # V^T GEMM (DoubleRow): V^T[m, vc] = sum_c h[c,m] Wv8[c,vc] / 8 + bv
            for m in range(8):
                j, slot = divmod(m, 2)
                if slot == 0:
                    vt_sb[b][j] = vtpool.tile([128, 1024], FP8, tag=f"vt{b}{j}",
                                              name=f"vt{b}{j}")
                pv = ps_o.tile([128, 512], F32, tag="o")
                mc = slice(m * 128, (m + 1) * 128)
                nc.tensor.matmul(pv[:, 0:256], lhsT=h2[:, :, mc], rhs=wv2[:],
                                 start=True, stop=True, perf_mode=DR)
                # vt 512-block layout per slot: [V0|1|1|V1][V2|1|1|V3] so each
                # head's 128-col block carries the all-ones Z columns.
                vt = vt_sb[b][j][:, 512 * slot:512 * slot + 512]
                vt4 = vt.rearrange("p (a u v d) -> p a u v d", a=2, u=2, v=2)
                pv4 = pv[:, 0:256].rearrange("p (a w d) -> p a w d", a=2, w=2)
                bvb4 = bvb[:].rearrange("p (a w d) -> p a w d", a=2, w=2)
                nc.vector.scalar_tensor_tensor(vt4[:, :, 0, 0, :], pv4[:, :, 0, :],
                                               QKV_SCALE, bvb4[:, :, 0, :], mult, add)
                nc.vector.scalar_tensor_tensor(vt4[:, :, 1, 1, :], pv4[:, :, 1, :],
                                               QKV_SCALE, bvb4[:, :, 1, :], mult, add)
                vtq = vt.rearrange("p (a q d) -> p a q d", a=2, q=4)
                nc.vector.tensor_copy(vtq[:, :, 1:3, :], ones_f32[:, 0:256].rearrange(
                    "p (a d) -> p a d", a=2).rearrange("p a (u d) -> p a u d", u=2))

        # xb = x + proj bias (first consumed by proj ~20us later; emitted
        # after ALL GEMM evacs so it never delays the first S matmuls)
        for b in range(BL):
            for ct in range(2):
                xbt = xbpool.tile([128, N], F32, tag=f"xb{b}{ct}", name=f"xb{b}{ct}")
                xb_sb[b][ct] = xbt
                nc.vector.tensor_scalar(xbt[:], x_sb[b][ct][:],
                                        bp_sb[:, ct:ct + 1], None, add)

        # ================= Phase B: attention + proj/residual ===============
        # nch-outer unit order so proj(nch) can fire after two units; the PE
        # executes its queue in order, so S matmuls are emitted PIPE steps
        # ahead of the exp-dependent AV matmuls.
        for b in range(BL):
            o_sb[b] = opool.tile([128, 2 * N], FP8, tag=f"o{b}", name=f"ot{b}")
        units = [(b, nch, p) for b in range(BL) for nch in range(2) for p in range(2)]
        seq = [(u, m) for u in range(len(units)) for m in range(8)]
        s_tiles = {}
        po_tiles = {}
        ex_pair = {}

        def emit_S(i):
            u, m = seq[i]
            b, nch, p = units[u]
            qt, kt = qk_sb[b][p], qk_sb[b][2 + p]
            ns = slice(nch * 512, (nch + 1) * 512)
            mc = slice(m * 128, (m + 1) * 128)
            ps = ps_big.tile([128, N], F32, tag="big", name="ps")
            nc.tensor.matmul(ps[:, 0:512], lhsT=kt[0:64, mc],
                             rhs=qt[0:64, ns], start=True, stop=True)
            nc.tensor.matmul(ps[:, 512:1024], lhsT=kt[64:128, mc],
                             rhs=qt[64:128, ns], start=True, stop=True)
            s_tiles[i] = ps

        def emit_proj(b, nch):
            # proj (DoubleRow over the o p-pair) + residual via x+bp.  Both ct
            # halves share ONE ps_big tile so only one S-pipeline slot is
            # borrowed, and the caller defers this into the NEXT unit's steps
            # so the S queue is already primed ahead of it.
            ns = slice(nch * 512, (nch + 1) * 512)
            o2 = o_sb[b][:].rearrange("p (s n) -> p s n", s=2)
            pp = ps_big.tile([128, N], F32, tag="big", name="pp")
            for ct in range(2):
                cs = slice(ct * 512, (ct + 1) * 512)
                nc.tensor.matmul(pp[:, cs],
                                 lhsT=wp2[:, :, ct * 128:(ct + 1) * 128],
                                 rhs=o2[:, :, ns], start=True, stop=True,
                                 perf_mode=DR)
            for ct in range(2):
                cs = slice(ct * 512, (ct + 1) * 512)
                outt = outpool.tile([128, 512], F32, tag="out")
                nc.vector.scalar_tensor_tensor(outt[:], pp[:, cs], QKV_SCALE,
                                               xb_sb[b][ct][:, ns], mult, add)
                nc.sync.dma_start(y_d[b, ct * 128:(ct + 1) * 128, ns], outt[:])

        PIPE = 3
        pending_proj = []
        for i in range(PIPE):
            emit_S(i)
        for i, (u, m) in enumerate(seq):
            if i + PIPE < len(seq):
                emit_S(i + PIPE)
            if pending_proj and seq[i][1] == 5:
                emit_proj(*pending_proj.pop(0))
            b, nch, p = units[u]
            ns = slice(nch * 512, (nch + 1) * 512)
            h0, h1 = 2 * p, 2 * p + 1
            if m == 0:
                po_tiles[u] = (
                    ps_o.tile([128, 512], F32, tag="o", name="po0"),
                    ps_o.tile([128, 512], F32, tag="o", name="po1"),
                )
            po0, po1 = po_tiles[u]
            ps = s_tiles.pop(i)
            j, slot = divmod(m, 2)
            if slot == 0:
                ex_pair[u] = expool.tile([128, 2 * N], FP8, tag="ex", name="ex")
            ext = ex_pair[u]
            # exp(S - 3): S bounded ~|8| so exp(S-3) <= e^5 fits fp8e4 (max
            # 448) while typical per-column maxima stay in the normal range.
            nc.scalar.activation(ext[:, N * slot:N * slot + N], ps[:], Exp,
                                 bias=neg3[:])
            if slot != 1:
                continue
            first, last = (j == 0), (j == 3)
            ex2 = ext[:].rearrange("p (s n) -> p s n", s=2)
            vt2 = vt_sb[b][j][:].rearrange("p (s c) -> p s c", s=2)
            # AV+Z DoubleRow over the m-chunk pair: [V_h0|1] -> O rows 0:64,
            # Zrep rows 64:128; [1|V_h1] mirrored.
            nc.tensor.matmul(
                po0[:], lhsT=vt2[:, :, 128 * h0:128 * h0 + 128],
                rhs=ex2[:, :, 0:512], start=first, stop=last, perf_mode=DR)
            nc.tensor.matmul(
                po1[:], lhsT=vt2[:, :, 128 * h1:128 * h1 + 128],
                rhs=ex2[:, :, 512:1024], start=first, stop=last, perf_mode=DR)
            if not last:
                continue
            # Evacuate each po bank with one full-tile copy so its PSUM slot
            # frees immediately (the Z-shift DMA round-trip would otherwise
            # hold it ~1.5us and stall the next unit's AV); normalize on SBUF.
            # reciprocal_approx_fast only ever runs at base partition 0.
            ot2 = o_sb[b][:].rearrange("p (s n) -> p s n", s=2)
            poc0 = rzpool.tile([128, 512], F32, tag="poc0")
            nc.vector.tensor_copy(poc0[:], po0[:])
            poc1 = rzpool.tile([128, 512], F32, tag="poc1")
            nc.vector.tensor_copy(poc1[:], po1[:])
            zs0 = rzpool.tile([64, 512], F32, tag="zs0")
            nc.sync.dma_start(zs0[:], poc0[64:128, :])
            rz1 = rzpool.tile([128, 512], F32, tag="rz", name="rz1")
            nc.vector.reciprocal_approx_fast(rz1[0:64, :], poc1[0:64, :])
            rzs1 = rzpool.tile([128, 512], F32, tag="rzs1")
            nc.sync.dma_start(rzs1[64:128, :], rz1[0:64, :])
            rzs0 = rzpool.tile([64, 512], F32, tag="rzs0")
            nc.vector.reciprocal_approx_fast(rzs0[:], zs0[:])
            nc.vector.tensor_mul(ot2[0:64, p, ns], poc0[0:64, :], rzs0[:])
            nc.vector.tensor_mul(ot2[64:128, p, ns], poc1[64:128, :],
                                 rzs1[64:128, :])
            if p == 1:
                pending_proj.append((b, nch))
        while pending_proj:
            emit_proj(*pending_proj.pop(0))

    nc.compile()
    return nc


def prep_inputs(x, gn_gamma, gn_beta, qkv_w, qkv_b, proj_w, proj_b):
    """Host-side weight prep shared by kernel() and the test harness."""
    x = np.ascontiguousarray(np.asarray(x, np.float32)).reshape(B, C, N)
    gn_gamma = np.asarray(gn_gamma, np.float32)
    gn_beta = np.asarray(gn_beta, np.float32)
    qkv_w = np.asarray(qkv_w, np.float32)
    qkv_b = np.asarray(qkv_b, np.float32)
    proj_w = np.asarray(proj_w, np.float32)
    proj_b = np.asarray(proj_b, np.float32)

    # fold GroupNorm affine into the qkv GEMM
    W3 = qkv_w * gn_gamma[None, :]
    b3 = qkv_b + qkv_w @ gn_beta
    W3r = W3.reshape(NH, 3, D, C)
    b3r = b3.reshape(NH, 3, D)
    scale = np.float32(D ** -0.5)
    Wq = W3r[:, 0].reshape(C, C)
    Wk = W3r[:, 1].reshape(C, C)          # d^-0.5 folded in the evac constant
    Wv = W3r[:, 2].reshape(C, C)
    bq = b3r[:, 0].reshape(C)
    bk = b3r[:, 1].reshape(C) * scale
    bv = b3r[:, 2].reshape(C)

    def pair_ct(wt):  # [256, out] -> [128, 2*out]: contraction split in 2 slots
        o = wt.shape[1]
        return np.ascontiguousarray(
            wt.reshape(2, 128, o).transpose(1, 0, 2).reshape(128, 2 * o))

    # weights x8 so fp8e4 quantization keeps ~0.5-scale values
    wqk8 = pair_ct((np.concatenate([Wq, Wk], axis=0).T * 8.0).astype(np.float32))
    wv8 = pair_ct((Wv.T * 8.0).astype(np.float32))
    wp8 = pair_ct((proj_w.T * 8.0).astype(np.float32))
    bqk = np.concatenate([bq, bk]).reshape(4, 128)
    bp2 = proj_b.reshape(2, 128)

    cidx = np.arange(128)
    gmap = np.zeros((128, 16), np.float32)
    gmap[cidx, cidx // 8] = 1.0 / 8.0
    gexp = np.zeros((16, 128), np.float32)
    gexp[cidx // 8, cidx] = 1.0

    common = {
        "wqk8": wqk8.astype(np.float32),
        "wv8": wv8.astype(np.float32),
        "wp8": wp8.astype(np.float32),
        "bqk": bqk.astype(np.float32),
        "bv": np.ascontiguousarray(bv[None, :], np.float32),
        "bp2": np.ascontiguousarray(bp2, np.float32),
        "gmap": gmap,
        "gexp": gexp,
    }
    in_maps = [
        {**common, "x": np.ascontiguousarray(x[c * BL:(c + 1) * BL])}
        for c in range(NCORES)
    ]
    return in_maps


_NC_CACHE = []


def kernel(x, gn_gamma, gn_beta, qkv_w, qkv_b, proj_w, proj_b, trace=False):
    in_maps = prep_inputs(x, gn_gamma, gn_beta, qkv_w, qkv_b, proj_w, proj_b)
    if not _NC_CACHE:
        _NC_CACHE.append(build_bass())
    nc = _NC_CACHE[0]
    res = run_bass_kernel_spmd(nc, in_maps, list(range(NCORES)), trace=trace)
    y = np.stack([res.results[c]["y"] for c in range(NCORES)])
    y = y.reshape(B, C, HH, WW)
    kernel.last_result = res
    return y


# revision 61
# speedup vs baseline: 1.1654x; 1.1654x over previous
"""AttentionBlock (GroupNorm + 4-head self-attention + proj + residual) on 8 trn2 cores.

Sharding: data-parallel over batch (B=16 -> 2 per core). Each core runs the full
block on its 2 batch elements; no collectives.

Device pipeline per batch (all layouts chosen so no on-device transposes are needed):
  - GroupNorm stats via bn_stats/bn_aggr + tiny PE matmuls for the cross-partition
    group combine (gamma/beta folded into the qkv weights on host); inv_std via a
    DVE bit-trick rsqrt + Newton so the ACT engine never leaves the Exp table.
  - h cast to fp8 (e4m3); qkv / V^T / proj GEMMs run fp8 DoubleRow (2 MAC/cell/cyc)
    with weights pre-scaled x8 on host for e4m3 range, compensated in the PSUM evac.
    Weight fp8 casts run on the ACT engine, which is idle until the first exp.
  - Q,K evac'd to f32r, so the S^T matmuls are full precision (their contraction
    is only d=64, so DoubleRow could not speed them up anyway).
  - Scores computed transposed: S^T[m,n] = K^T Q; softmax's sum runs over the
    PSUM partition axis via all-ones stationary columns packed next to V.
  - exp(S-3) on ACT straight out of PSUM into fp8 m-chunk-pair tiles (S bounded
    ~|8|; the uniform e^-3 scale cancels in the softmax normalization).
  - AV+Z as fp8 DoubleRow over m-chunk pairs: [V_h|1] stationary emits O and the
    denominator Z replicated across partitions in one pass.
  - Normalization reads O and Z directly from PSUM (reciprocal at base partition
    0 only -- reciprocal_approx_fast mis-executes at base 64); Z moved across
    partitions by small SBUF-SBUF DMAs; output quantized to fp8 p-pair tiles.
  - proj fp8 DoubleRow per nch half (starts after 2 of 4 units) + residual from
    x with the proj bias pre-added on the DVE (no rank-1 bias matmuls).
"""

import numpy as np
from contextlib import ExitStack

import concourse.bass as bass
import concourse.bacc as bacc
import concourse.tile as tile
import concourse.mybir as mybir
from concourse.bass_utils import run_bass_kernel_spmd

F32 = mybir.dt.float32
F32R = mybir.dt.float32r
BF16 = mybir.dt.bfloat16
FP8 = mybir.dt.float8e4
I32 = mybir.dt.int32
DR = mybir.MatmulPerfMode.DoubleRow

B, C, HH, WW = 16, 256, 32, 32
N = HH * WW           # 1024 spatial positions
NH = 4                # heads
D = C // NH           # 64 head dim
G = 32                # groups
EPS = 1e-5
NCORES = 8
BL = B // NCORES      # batches per core

RSQRT_MAGIC = 0x5F3759DF
QKV_SCALE = 0.125             # weights stored x8
K_SCALE = 0.125 * 0.125       # extra d^-0.5 fold for K


def build_bass():
    nc = bacc.Bacc("TRN2", target_bir_lowering=False, debug=False)

    x_d = nc.dram_tensor("x", [BL, C, N], F32, kind="ExternalInput").ap()
    wqk_d = nc.dram_tensor("wqk8", [128, 1024], F32, kind="ExternalInput").ap()
    wv_d = nc.dram_tensor("wv8", [128, 512], F32, kind="ExternalInput").ap()
    wp_d = nc.dram_tensor("wp8", [128, 512], F32, kind="ExternalInput").ap()
    bqk_d = nc.dram_tensor("bqk", [4, 128], F32, kind="ExternalInput").ap()
    bv_d = nc.dram_tensor("bv", [1, 256], F32R, kind="ExternalInput").ap()
    bp_d = nc.dram_tensor("bp2", [2, 128], F32, kind="ExternalInput").ap()
    gmap_d = nc.dram_tensor("gmap", [128, 16], F32, kind="ExternalInput").ap()
    gexp_d = nc.dram_tensor("gexp", [16, 128], F32, kind="ExternalInput").ap()
    y_d = nc.dram_tensor("y", [BL, C, N], F32, kind="ExternalOutput").ap()

    Exp = mybir.ActivationFunctionType.Exp
    Ident = mybir.ActivationFunctionType.Identity
    mult = mybir.AluOpType.mult
    sub = mybir.AluOpType.subtract
    add = mybir.AluOpType.add
    asr = mybir.AluOpType.arith_shift_right

    with tile.TileContext(nc) as tc, ExitStack() as ctx:
        consts = ctx.enter_context(tc.tile_pool(name="consts", bufs=1))
        xpool = ctx.enter_context(tc.tile_pool(name="xp", bufs=1))
        xbpool = ctx.enter_context(tc.tile_pool(name="xbp", bufs=1))
        hpool = ctx.enter_context(tc.tile_pool(name="hp", bufs=1))
        qkpool = ctx.enter_context(tc.tile_pool(name="qkp", bufs=1))
        vtpool = ctx.enter_context(tc.tile_pool(name="vtp", bufs=1))
        opool = ctx.enter_context(tc.tile_pool(name="op", bufs=1))
        gnpool = ctx.enter_context(tc.tile_pool(name="gnp", bufs=2))
        expool = ctx.enter_context(tc.tile_pool(name="exp", bufs=4))
        rzpool = ctx.enter_context(tc.tile_pool(name="rzp", bufs=3))
        outpool = ctx.enter_context(tc.tile_pool(name="outp", bufs=4))
        ps_big = ctx.enter_context(tc.tile_pool(name="psb", bufs=3, space="PSUM"))
        ps_o = ctx.enter_context(tc.tile_pool(name="pso", bufs=2, space="PSUM"))

        x_sb = [[None] * 2 for _ in range(BL)]
        xb_sb = [[None] * 2 for _ in range(BL)]
        h_sb = [None] * BL            # fp8 ct-pair tiles [128, 2*1024]
        qk_sb = [[None] * 4 for _ in range(BL)]   # f32r: q01 q23 k01 k23
        vt_sb = [[None] * 4 for _ in range(BL)]   # fp8 m-chunk pair tiles
        o_sb = [None] * BL            # fp8 p-pair tiles [128, 2*1024]

        # --- x first (GroupNorm gates everything), split for earlier stats ---
        for b in range(BL):
            for ct in range(2):
                xt = xpool.tile([128, N], F32, tag=f"x{b}{ct}", name=f"x{b}{ct}")
                x_sb[b][ct] = xt
                nc.sync.dma_start(xt[:, 0:512], x_d[b, ct * 128:(ct + 1) * 128, 0:512])
                nc.sync.dma_start(xt[:, 512:1024], x_d[b, ct * 128:(ct + 1) * 128, 512:1024])

        # --- constants / weights into SBUF (fp8 casts on the idle ACT) ------
        gmap_sb = consts.tile([128, 16], F32, tag="gmap")
        nc.sync.dma_start(gmap_sb[:], gmap_d[:])
        gexp_sb = consts.tile([16, 128], F32, tag="gexp")
        nc.sync.dma_start(gexp_sb[:], gexp_d[:])
        bqk_sb = consts.tile([128, 4], F32, tag="bqk")
        nc.sync.dma_start(bqk_sb[:], bqk_d.transpose([1, 0]))
        bp_sb = consts.tile([128, 2], F32, tag="bp")
        nc.sync.dma_start(bp_sb[:], bp_d.transpose([1, 0]))
        bv_sb = consts.tile([1, 256], F32R, tag="bv")
        nc.sync.dma_start(bv_sb[:], bv_d[:])
        wqk_f = consts.tile([128, 1024], F32, tag="wqk_f")
        nc.sync.dma_start(wqk_f[:, 0:512], wqk_d[:, 0:512])
        nc.sync.dma_start(wqk_f[:, 512:1024], wqk_d[:, 512:1024])
        wv_f = consts.tile([128, 512], F32, tag="wv_f")
        nc.sync.dma_start(wv_f[:], wv_d[:])
        wp_f = consts.tile([128, 512], F32, tag="wp_f")
        nc.sync.dma_start(wp_f[:], wp_d[:])

        ones_f32 = consts.tile([128, 512], F32, tag="ones_f32")
        nc.vector.memset(ones_f32[:], 1.0)
        neg3 = consts.tile([128, 1], F32, tag="neg3")
        nc.vector.memset(neg3[:], -3.0)
        ones_sb = consts.tile([128, 512], F32R, tag="ones")
        nc.vector.tensor_copy(ones_sb[:], ones_f32[:])
        wqk8 = consts.tile([128, 1024], FP8, tag="wqk8")
        nc.scalar.copy(wqk8[:], wqk_f[:])
        wv8 = consts.tile([128, 512], FP8, tag="wv8")
        nc.scalar.copy(wv8[:], wv_f[:])
        wp8 = consts.tile([128, 512], FP8, tag="wp8")
        nc.scalar.copy(wp8[:], wp_f[:])
        wqk2 = wqk8[:].rearrange("p (s o) -> p s o", s=2)
        wv2 = wv8[:].rearrange("p (s o) -> p s o", s=2)
        wp2 = wp8[:].rearrange("p (s o) -> p s o", s=2)

        # bv broadcast to all partitions via a rank-1 matmul (done once)
        pbv = ps_o.tile([128, 256], F32, tag="o", name="pbv")
        nc.tensor.matmul(pbv[:], lhsT=ones_sb[0:1, 0:128], rhs=bv_sb[0:1, :],
                         start=True, stop=True)
        bvb = consts.tile([128, 256], F32, tag="bvb")
        nc.scalar.copy(bvb[:], pbv[:])

        # ====== Phase A: GroupNorm, batched 4-wide across (b, ct) tiles ======
        units4 = [(b, ct) for b in range(BL) for ct in range(2)]
        bn6 = gnpool.tile([128, 48], F32, tag="bn6")
        mva = gnpool.tile([128, 8], F32, tag="mva")  # (mean, var) x 4 units
        for u4, (b, ct) in enumerate(units4):
            xt = x_sb[b][ct]
            nc.vector.bn_stats(bn6[:, 12 * u4:12 * u4 + 6], xt[:, 0:512])
            nc.vector.bn_stats(bn6[:, 12 * u4 + 6:12 * u4 + 12], xt[:, 512:1024])
            nc.vector.bn_aggr(mva[:, 2 * u4:2 * u4 + 2], bn6[:, 12 * u4:12 * u4 + 12])
        mva2 = mva[:].rearrange("p (u c) -> p u c", c=2)
        m2a = gnpool.tile([128, 4], F32, tag="m2a")
        nc.vector.tensor_mul(m2a[:], mva2[:, :, 0], mva2[:, :, 0])
        # group-combine via matmul, then a short DVE-only chain:
        # s = rsqrt(var+eps) via bit trick + one Newton step; t = mean*s.
        psg = ps_o.tile([16, 12], F32, tag="o", name="psg")
        nc.tensor.matmul(psg[:, 0:8], lhsT=gmap_sb[:], rhs=mva[:], start=True,
                         stop=True, skip_group_check=True)
        nc.tensor.matmul(psg[:, 8:12], lhsT=gmap_sb[:], rhs=m2a[:], start=True,
                         stop=True, skip_group_check=True)
        g = gnpool.tile([16, 20], F32, tag="g")
        nc.vector.tensor_copy(g[:, 0:12], psg[:])
        g2 = g[:, 0:8].rearrange("p (u c) -> p u c", c=2)
        # var_g + eps = (E[var] + eps) + E[mean^2] - mean_g^2
        nc.vector.scalar_tensor_tensor(g[:, 12:16], g2[:, :, 1], EPS,
                                       g[:, 8:12], add, add)
        nc.vector.tensor_mul(g[:, 16:20], g2[:, :, 0], g2[:, :, 0])
        nc.vector.tensor_sub(g[:, 12:16], g[:, 12:16], g[:, 16:20])
        gi = g[:].bitcast(I32)
        nc.vector.tensor_scalar(gi[:, 16:20], gi[:, 12:16], 1, None, asr)
        nc.vector.tensor_scalar(gi[:, 16:20], gi[:, 16:20], -1, RSQRT_MAGIC,
                                mult, add)
        sg = gnpool.tile([16, 8], F32, tag="sg")
        s2 = sg[:].rearrange("p (u c) -> p u c", c=2)
        nc.vector.tensor_mul(g[:, 8:12], g[:, 16:20], g[:, 16:20])
        nc.vector.tensor_mul(g[:, 8:12], g[:, 8:12], g[:, 12:16])
        nc.vector.tensor_scalar(g[:, 8:12], g[:, 8:12], -0.5, 1.5, mult, add)
        nc.vector.tensor_mul(s2[:, :, 0], g[:, 16:20], g[:, 8:12])
        nc.vector.tensor_mul(s2[:, :, 1], g2[:, :, 0], s2[:, :, 0])
        psc = ps_o.tile([128, 8], F32, tag="o", name="psc")
        nc.tensor.matmul(psc[:], lhsT=gexp_sb[:], rhs=sg[:], start=True, stop=True)
        st = gnpool.tile([128, 8], F32, tag="st")
        nc.vector.tensor_copy(st[:], psc[:])
        # PE warm-up: the clock needs ~3.4us of sustained activity; these run
        # while the DVE finishes the GN chain + h applies.
        for w in range(14):
            nc.tensor.matmul(pbv[:], lhsT=ones_sb[0:1, 0:128],
                             rhs=bv_sb[0:1, :], start=True, stop=True)
        def emit_h(b):
            # h = x * s - t  (gamma/beta already folded into W/b on host);
            # per batch so batch 1's applies don't delay batch 0's GEMMs
            h_sb[b] = hpool.tile([128, 2 * N], FP8, tag=f"h{b}", name=f"h{b}")
            for ct in range(2):
                u4 = 2 * b + ct
                nc.vector.tensor_scalar(h_sb[b][:, N * ct:N * ct + N],
                                        x_sb[b][ct][:],
                                        st[:, 2 * u4:2 * u4 + 1],
                                        st[:, 2 * u4 + 1:2 * u4 + 2], mult, sub)

        emit_h(0)
        for b in range(BL):
            if b == 1:
                emit_h(1)
            h2 = h_sb[b][:].rearrange("p (s n) -> p s n", s=2)
            # Q,K GEMM (fp8 DoubleRow over the 2 c-blocks): ot 0=q01 1=q23
            # 2=k01 3=k23; the evac rescales the x8 weights and adds the bias.
            for ot in (2, 0, 3, 1):
                pq = ps_big.tile([128, N], F32, tag="big")
                for nch in range(2):
                    ns = slice(nch * 512, (nch + 1) * 512)
                    nc.tensor.matmul(pq[:, ns],
                                     lhsT=wqk2[:, :, ot * 128:(ot + 1) * 128],
                                     rhs=h2[:, :, ns], start=True, stop=True,
                                     perf_mode=DR)
                qk = qkpool.tile([128, N], F32R, tag=f"qk{b}{ot}")
                qk_sb[b][ot] = qk
                sc = K_SCALE if ot >= 2 else QKV_SCALE
                if b == 0:
                    # batch 0's evacs on the (still idle) ACT engine so the
                    # DVE builds vt tiles in parallel -> earlier first S;
                    # Identity shares the Exp table (no table reload).
                    nc.scalar.activation(qk[:], pq[:], Ident,
                                         bias=bqk_sb[:, ot:ot + 1], scale=sc)
                else:
                    nc.vector.tensor_scalar(qk[:], pq[:], sc,
                                            bqk_sb[:, ot:ot + 1], mult, add)

            # V^T GEMM (DoubleRow): V^T[m, vc] = sum_c h[c,m] Wv8[c,vc] / 8 + bv
            for m in range(8):
                j, slot = divmod(m, 2)
                if slot == 0:
                    vt_sb[b][j] = vtpool.tile([128, 1024], FP8, tag=f"vt{b}{j}",
                                              name=f"vt{b}{j}")
                pv = ps_o.tile([128, 512], F32, tag="o")
                mc = slice(m * 128, (m + 1) * 128)
                nc.tensor.matmul(pv[:, 0:256], lhsT=h2[:, :, mc], rhs=wv2[:],
                                 start=True, stop=True, perf_mode=DR)
                # vt 512-block layout per slot: [V0|1|1|V1][V2|1|1|V3] so each
                # head's 128-col block carries the all-ones Z columns.
                vt = vt_sb[b][j][:, 512 * slot:512 * slot + 512]
                vt4 = vt.rearrange("p (a u v d) -> p a u v d", a=2, u=2, v=2)
                pv4 = pv[:, 0:256].rearrange("p (a w d) -> p a w d", a=2, w=2)
                bvb4 = bvb[:].rearrange("p (a w d) -> p a w d", a=2, w=2)
                nc.vector.scalar_tensor_tensor(vt4[:, :, 0, 0, :], pv4[:, :, 0, :],
                                               QKV_SCALE, bvb4[:, :, 0, :], mult, add)
                nc.vector.scalar_tensor_tensor(vt4[:, :, 1, 1, :], pv4[:, :, 1, :],
                                               QKV_SCALE, bvb4[:, :, 1, :], mult, add)
                vtq = vt.rearrange("p (a q d) -> p a q d", a=2, q=4)
                nc.vector.tensor_copy(vtq[:, :, 1:3, :], ones_f32[:, 0:256].rearrange(
                    "p (a d) -> p a d", a=2).rearrange("p a (u d) -> p a u d", u=2))

        # xb = x + proj bias (first consumed by proj ~20us later; emitted
        # after ALL GEMM evacs so it never delays the first S matmuls)
        for b in range(BL):
            for ct in range(2):
                xbt = xbpool.tile([128, N], F32, tag=f"xb{b}{ct}", name=f"xb{b}{ct}")
                xb_sb[b][ct] = xbt
                nc.vector.tensor_scalar(xbt[:], x_sb[b][ct][:],
                                        bp_sb[:, ct:ct + 1], None, add)

        # ================= Phase B: attention + proj/residual ===============
        # nch-outer unit order so proj(nch) can fire after two units; the PE
        # executes its queue in order, so S matmuls are emitted PIPE steps
        # ahead of the exp-dependent AV matmuls.
        for b in range(BL):
            o_sb[b] = opool.tile([128, 2 * N], FP8, tag=f"o{b}", name=f"ot{b}")
        units = [(b, nch, p) for b in range(BL) for nch in range(2) for p in range(2)]
        seq = [(u, m) for u in range(len(units)) for m in range(8)]
        s_tiles = {}
        po_tiles = {}
        ex_pair = {}

        def emit_S(i):
            u, m = seq[i]
            b, nch, p = units[u]
            qt, kt = qk_sb[b][p], qk_sb[b][2 + p]
            ns = slice(nch * 512, (nch + 1) * 512)
            mc = slice(m * 128, (m + 1) * 128)
            ps = ps_big.tile([128, N], F32, tag="big", name="ps")
            nc.tensor.matmul(ps[:, 0:512], lhsT=kt[0:64, mc],
                             rhs=qt[0:64, ns], start=True, stop=True)
            nc.tensor.matmul(ps[:, 512:1024], lhsT=kt[64:128, mc],
                             rhs=qt[64:128, ns], start=True, stop=True)
            s_tiles[i] = ps

        def emit_proj(b, nch):
            # proj (DoubleRow over the o p-pair) + residual via x+bp.  Both ct
            # halves share ONE ps_big tile so only one S-pipeline slot is
            # borrowed, and the caller defers this into the NEXT unit's steps
            # so the S queue is already primed ahead of it.
            ns = slice(nch * 512, (nch + 1) * 512)
            o2 = o_sb[b][:].rearrange("p (s n) -> p s n", s=2)
            pp = ps_big.tile([128, N], F32, tag="big", name="pp")
            for ct in range(2):
                cs = slice(ct * 512, (ct + 1) * 512)
                nc.tensor.matmul(pp[:, cs],
                                 lhsT=wp2[:, :, ct * 128:(ct + 1) * 128],
                                 rhs=o2[:, :, ns], start=True, stop=True,
                                 perf_mode=DR)
            for ct in range(2):
                cs = slice(ct * 512, (ct + 1) * 512)
                outt = outpool.tile([128, 512], F32, tag="out")
                nc.vector.scalar_tensor_tensor(outt[:], pp[:, cs], QKV_SCALE,
                                               xb_sb[b][ct][:, ns], mult, add)
                nc.sync.dma_start(y_d[b, ct * 128:(ct + 1) * 128, ns], outt[:])

        PIPE = 3
        pending_proj = []
        for i in range(PIPE):
            emit_S(i)
        for i, (u, m) in enumerate(seq):
            if i + PIPE < len(seq):
                emit_S(i + PIPE)
            if pending_proj and seq[i][1] == 5:
                emit_proj(*pending_proj.pop(0))
            b, nch, p = units[u]
            ns = slice(nch * 512, (nch + 1) * 512)
            h0, h1 = 2 * p, 2 * p + 1
            if m == 0:
                po_tiles[u] = (
                    ps_o.tile([128, 512], F32, tag="o", name="po0"),
                    ps_o.tile([128, 512], F32, tag="o", name="po1"),
                )
            po0, po1 = po_tiles[u]
            ps = s_tiles.pop(i)
            j, slot = divmod(m, 2)
            if slot == 0:
                ex_pair[u] = expool.tile([128, 2 * N], FP8, tag="ex", name="ex")
            ext = ex_pair[u]
            # exp(S - 3): S bounded ~|8| so exp(S-3) <= e^5 fits fp8e4 (max
            # 448) while typical per-column maxima stay in the normal range.
            nc.scalar.activation(ext[:, N * slot:N * slot + N], ps[:], Exp,
                                 bias=neg3[:])
            if slot != 1:
                continue
            first, last = (j == 0), (j == 3)
            ex2 = ext[:].rearrange("p (s n) -> p s n", s=2)
            vt2 = vt_sb[b][j][:].rearrange("p (s c) -> p s c", s=2)
            # AV+Z DoubleRow over the m-chunk pair: [V_h0|1] -> O rows 0:64,
            # Zrep rows 64:128; [1|V_h1] mirrored.
            nc.tensor.matmul(
                po0[:], lhsT=vt2[:, :, 128 * h0:128 * h0 + 128],
                rhs=ex2[:, :, 0:512], start=first, stop=last, perf_mode=DR)
            nc.tensor.matmul(
                po1[:], lhsT=vt2[:, :, 128 * h1:128 * h1 + 128],
                rhs=ex2[:, :, 512:1024], start=first, stop=last, perf_mode=DR)
            if not last:
                continue
            # Evacuate each po bank with one full-tile copy so its PSUM slot
            # frees immediately (the Z-shift DMA round-trip would otherwise
            # hold it ~1.5us and stall the next unit's AV); normalize on SBUF.
            # reciprocal_approx_fast only ever runs at base partition 0.
            ot2 = o_sb[b][:].rearrange("p (s n) -> p s n", s=2)
            poc0 = rzpool.tile([128, 512], F32, tag="poc0")
            nc.vector.tensor_copy(poc0[:], po0[:])
            poc1 = rzpool.tile([128, 512], F32, tag="poc1")
            nc.vector.tensor_copy(poc1[:], po1[:])
            zs0 = rzpool.tile([64, 512], F32, tag="zs0")
            nc.sync.dma_start(zs0[:], poc0[64:128, :])
            rz1 = rzpool.tile([128, 512], F32, tag="rz", name="rz1")
            nc.vector.reciprocal_approx_fast(rz1[0:64, :], poc1[0:64, :])
            rzs1 = rzpool.tile([128, 512], F32, tag="rzs1")
            nc.sync.dma_start(rzs1[64:128, :], rz1[0:64, :])
            rzs0 = rzpool.tile([64, 512], F32, tag="rzs0")
            nc.vector.reciprocal_approx_fast(rzs0[:], zs0[:])
            nc.vector.tensor_mul(ot2[0:64, p, ns], poc0[0:64, :], rzs0[:])
            nc.vector.tensor_mul(ot2[64:128, p, ns], poc1[64:128, :],
                                 rzs1[64:128, :])
            if p == 1:
                pending_proj.append((b, nch))
        while pending_proj:
            emit_proj(*pending_proj.pop(0))

    nc.compile()
    return nc


def prep_inputs(x, gn_gamma, gn_beta, qkv_w, qkv_b, proj_w, proj_b):
    """Host-side weight prep shared by kernel() and the test harness."""
    x = np.ascontiguousarray(np.asarray(x, np.float32)).reshape(B, C, N)
    gn_gamma = np.asarray(gn_gamma, np.float32)
    gn_beta = np.asarray(gn_beta, np.float32)
    qkv_w = np.asarray(qkv_w, np.float32)
    qkv_b = np.asarray(qkv_b, np.float32)
    proj_w = np.asarray(proj_w, np.float32)
    proj_b = np.asarray(proj_b, np.float32)

    # fold GroupNorm affine into the qkv GEMM
    W3 = qkv_w * gn_gamma[None, :]
    b3 = qkv_b + qkv_w @ gn_beta
    W3r = W3.reshape(NH, 3, D, C)
    b3r = b3.reshape(NH, 3, D)
    scale = np.float32(D ** -0.5)
    Wq = W3r[:, 0].reshape(C, C)
    Wk = W3r[:, 1].reshape(C, C)          # d^-0.5 folded in the evac constant
    Wv = W3r[:, 2].reshape(C, C)
    bq = b3r[:, 0].reshape(C)
    bk = b3r[:, 1].reshape(C) * scale
    bv = b3r[:, 2].reshape(C)

    def pair_ct(wt):  # [256, out] -> [128, 2*out]: contraction split in 2 slots
        o = wt.shape[1]
        return np.ascontiguousarray(
            wt.reshape(2, 128, o).transpose(1, 0, 2).reshape(128, 2 * o))

    # weights x8 so fp8e4 quantization keeps ~0.5-scale values
    wqk8 = pair_ct((np.concatenate([Wq, Wk], axis=0).T * 8.0).astype(np.float32))
    wv8 = pair_ct((Wv.T * 8.0).astype(np.float32))
    wp8 = pair_ct((proj_w.T * 8.0).astype(np.float32))
    bqk = np.concatenate([bq, bk]).reshape(4, 128)
    bp2 = proj_b.reshape(2, 128)

    cidx = np.arange(128)
    gmap = np.zeros((128, 16), np.float32)
    gmap[cidx, cidx // 8] = 1.0 / 8.0
    gexp = np.zeros((16, 128), np.float32)
    gexp[cidx // 8, cidx] = 1.0

    common = {
        "wqk8": wqk8.astype(np.float32),
        "wv8": wv8.astype(np.float32),
        "wp8": wp8.astype(np.float32),
        "bqk": bqk.astype(np.float32),
        "bv": np.ascontiguousarray(bv[None, :], np.float32),
        "bp2": np.ascontiguousarray(bp2, np.float32),
        "gmap": gmap,
        "gexp": gexp,
    }
    in_maps = [
        {**common, "x": np.ascontiguousarray(x[c * BL:(c + 1) * BL])}
        for c in range(NCORES)
    ]
    return in_maps


_NC_CACHE = []


def kernel(x, gn_gamma, gn_beta, qkv_w, qkv_b, proj_w, proj_b, trace=False):
    in_maps = prep_inputs(x, gn_gamma, gn_beta, qkv_w, qkv_b, proj_w, proj_b)
    if not _NC_CACHE:
        _NC_CACHE.append(build_bass())
    nc = _NC_CACHE[0]
    res = run_bass_kernel_spmd(nc, in_maps, list(range(NCORES)), trace=trace)
    y = np.stack([res.results[c]["y"] for c in range(NCORES)])
    y = y.reshape(B, C, HH, WW)
    kernel.last_result = res
    return y
